# revision 9
# baseline (speedup 1.0000x reference)
"""LocalMHA (windowed attention, window=128, look_backward=1, RoPE) on 8 TRN2 cores.

Sharding: sequence-parallel, no collectives. Core c handles batch c//2,
sequence half c%2 (2048 query tokens + a 128-token look-backward halo whose
x rows ride along in the core's input shard; zeros at a true sequence start,
where the mask kills the backward keys anyway).

Layout trick: within each 128-row (2-head) block of the head-transposed q/k,
rows are permuted to [hA_d0-31 | hB_d0-31 | hA_d32-63 | hB_d32-63] (host-side
column permutation of W_qkv). The rotate_half partner is then r^64, so RoPE
needs only full-width partition-shifted multiplies (no 32-row fragments), with
the sin sign folded host-side. Scores contract each head's d over two 32-row
chunks (PSUM accumulation) — the dot product is invariant to the d-permutation.

Dtypes: projections and scores in fp32r (full PE rate at free>=256, ~1.6e-4);
attention probabilities and v in bf16 (free-dim-128 matmuls at full rate).

Engine split: PE matmuls/transposes; DVE psum-side elementwise (RoPE muls,
mask-add, evictions); GPSIMD sbuf-side elementwise (RoPE add, normalize);
ACT exp(+row-sum accumulate) and psum evictions that DVE can't absorb.
"""

import numpy as np
from contextlib import ExitStack

import concourse.bacc as bacc
import concourse.tile as tile
import concourse.mybir as mybir
from concourse.bass_utils import run_bass_kernel_spmd
from concourse.masks import make_identity

# Problem shape (hardcoded per contract)
B, N, D = 4, 4096, 1024
H, DH, WS = 16, 64, 128
THETA = 10000.0
N3 = 3 * H * DH            # 3072
NCORES = 8
HALF = N // 2              # 2048 query tokens per core
NT = HALF + WS             # 2176 tokens incl halo window
NWIN = HALF // WS          # 16 query windows
SCALE = DH ** -0.5
NEG = -1.0e9

F32 = mybir.dt.float32
F32R = mybir.dt.float32r
BF16 = mybir.dt.bfloat16
ADD = mybir.AluOpType.add
MUL = mybir.AluOpType.mult
EXP = mybir.ActivationFunctionType.Exp

# token chunks for phases A/B (start, len); 128-aligned, len<=512
CHUNKS = [(0, 512), (512, 512), (1024, 512), (1536, 512), (2048, 128)]


def _rope(nc, tmpp, src_psum, dst, L, rp, ci, si):
    """dst[:, :L] = src*cos + rot64(src)*sin_signed, straight from PSUM.

    Permuted layout: rotate partner of row r is r^64. The sin tile is indexed
    by SOURCE row with the destination's sign folded in host-side, so each
    multiply's two inputs share a base partition (only outputs are shifted).
    The final add runs on GPSIMD (SBUF-only engine), via a plain-f32 view of
    the f32r tile (raw bits through DMA are accepted by the fp32r matmul).
    """
    t1 = tmpp.tile([128, 512], F32, tag="t1")
    nc.vector.tensor_tensor(t1[:, :L], src_psum[:, :L], rp[:, ci, :L], MUL)
    t2 = tmpp.tile([128, 512], F32, tag="t2")
    nc.vector.tensor_tensor(t2[0:64, :L], src_psum[64:128, :L],
                            rp[64:128, si, :L], MUL)
    nc.vector.tensor_tensor(t2[64:128, :L], src_psum[0:64, :L],
                            rp[0:64, si, :L], MUL)
    nc.gpsimd.tensor_tensor(dst[:, :L].bitcast(F32), t1[:, :L], t2[:, :L], ADD)


def _build(reps=1):
    nc = bacc.Bacc("TRN2", target_bir_lowering=False, debug=False,
                   enable_asserts=False, num_devices=NCORES)

    xs = nc.dram_tensor("xs", [NT, D], F32R, kind="ExternalInput").ap()
    wqkv = nc.dram_tensor("wqkv", [D, N3], F32R, kind="ExternalInput").ap()
    wout = nc.dram_tensor("wout", [D, D], F32R, kind="ExternalInput").ap()
    # 0:qcos 1:qsin 2:kcos_cur 3:ksin_cur 4:kcos_prev 5:ksin_prev
    ropes = nc.dram_tensor("ropes", [6, 128, 512], F32, kind="ExternalInput").ap()
    masks = nc.dram_tensor("masks", [2, 128, 256], F32, kind="ExternalInput").ap()
    out = nc.dram_tensor("out", [HALF, D], F32, kind="ExternalOutput").ap()

    # internal DRAM staging
    qrope = nc.dram_tensor("qrope", [D, HALF], F32R).ap()
    k2 = nc.dram_tensor("k2", [D, NWIN, 2 * WS], F32R).ap()
    vstage = nc.dram_tensor("vstage", [NT, D], BF16).ap()

    with tile.TileContext(nc) as tc:
        with ExitStack() as top:
            constp = top.enter_context(tc.tile_pool(name="const", bufs=1))
            identf = constp.tile([128, 128], F32, tag="idf")
            make_identity(nc, identf[:])
            identb = constp.tile([128, 128], BF16, tag="idb")
            nc.vector.tensor_copy(identb[:], identf[:])
            identr = constp.tile([128, 128], F32R, tag="idr")
            nc.vector.tensor_copy(identr[:], identf[:])
            rp = constp.tile([128, 6, 512], F32, tag="ropes")
            nc.sync.dma_start(rp[:], ropes.rearrange("r p m -> p r m"))
            mk = constp.tile([128, 2, 256], F32, tag="masks")
            nc.sync.dma_start(mk[:], masks.rearrange("r p m -> p r m"))

            rep_ctx = tc.For_i(0, reps, 1) if reps > 1 else ExitStack()
            top.enter_context(rep_ctx)

            # ---------------- Phase A+B: transpose + QKV + RoPE ----------------
            with ExitStack() as ab:
                wp = ab.enter_context(tc.tile_pool(name="wq", bufs=1))
                w_sb = wp.tile([128, 8, N3], F32R, tag="w")
                nc.sync.dma_start(w_sb[:],
                                  wqkv.rearrange("(c p) n -> p c n", p=128))
                xp = ab.enter_context(tc.tile_pool(name="xst", bufs=2))
                xtp = ab.enter_context(tc.tile_pool(name="xT", bufs=2))
                tmpp = ab.enter_context(tc.tile_pool(name="tmp", bufs=3))
                rop = ab.enter_context(tc.tile_pool(name="ro", bufs=4))
                vp = ab.enter_context(tc.tile_pool(name="vsb", bufs=3))
                tps = ab.enter_context(tc.tile_pool(name="tps", bufs=3, space="PSUM"))
                mps = ab.enter_context(tc.tile_pool(name="mps", bufs=4, space="PSUM"))

                for (s, L) in CHUNKS:
                    nmt = L // 128
                    xT = xtp.tile([128, 8, 512], F32R, tag="xT")
                    for mt in range(nmt):
                        xst = xp.tile([128, D], F32R, tag="x")
                        nc.sync.dma_start(xst[:], xs[s + mt * 128: s + (mt + 1) * 128, :])
                        for kc in range(8):
                            tp = tps.tile([128, 128], F32R, tag="tp")
                            nc.tensor.transpose(tp[:], xst[:, kc * 128:(kc + 1) * 128],
                                                identr[:])
                            nc.scalar.copy(xT[:, kc, mt * 128:(mt + 1) * 128], tp[:])

                    # q^T (+rope) for query tokens of this chunk
                    qs = 128 if s == 0 else 0
                    qL = L - qs
                    if qL > 0:
                        for nch in range(8):
                            qp = mps.tile([128, 512], F32, tag="mm")
                            for kc in range(8):
                                nc.tensor.matmul(qp[:, :qL],
                                                 w_sb[:, kc, nch * 128:(nch + 1) * 128],
                                                 xT[:, kc, qs:qs + qL],
                                                 start=(kc == 0), stop=(kc == 7))
                            qf = rop.tile([128, 512], F32R, tag="ro")
                            _rope(nc, tmpp, qp, qf, qL, rp, 0, 1)
                            q0 = s + qs - 128
                            nc.sync.dma_start(
                                qrope[nch * 128:(nch + 1) * 128, q0:q0 + qL],
                                qf[:, :qL])

                    # k^T with both rope phases
                    for nch in range(8):
                        kp = mps.tile([128, 512], F32, tag="mm")
                        for kc in range(8):
                            nc.tensor.matmul(kp[:, :L],
                                             w_sb[:, kc, 1024 + nch * 128: 1024 + (nch + 1) * 128],
                                             xT[:, kc, 0:L],
                                             start=(kc == 0), stop=(kc == 7))
                        cs = 128 if s == 0 else 0     # halo window has no cur slot
                        if L - cs > 0:
                            kcur = rop.tile([128, 512], F32R, tag="ro")
                            _rope(nc, tmpp, kp, kcur, L, rp, 2, 3)
                            w0 = (s + cs) // 128 - 1
                            nw = (L - cs) // 128
                            nc.sync.dma_start(
                                k2[nch * 128:(nch + 1) * 128, w0:w0 + nw, 128:256],
                                kcur[:, cs:L].rearrange("p (w i) -> p w i", i=128))
                        if s + L <= HALF:             # last window has no next
                            kprv = rop.tile([128, 512], F32R, tag="ro")
                            _rope(nc, tmpp, kp, kprv, L, rp, 4, 5)
                            w0 = s // 128
                            nw = L // 128
                            nc.sync.dma_start(
                                k2[nch * 128:(nch + 1) * 128, w0:w0 + nw, 0:128],
                                kprv[:, 0:L].rearrange("p (w i) -> p w i", i=128))

                    # v in natural layout, bf16
                    for mt in range(nmt):
                        vsb = vp.tile([128, D], BF16, tag="v")
                        for nh in range(2):
                            vq = mps.tile([128, 512], F32, tag="mm")
                            for kc in range(8):
                                nc.tensor.matmul(vq[:],
                                                 xT[:, kc, mt * 128:(mt + 1) * 128],
                                                 w_sb[:, kc, 2048 + nh * 512: 2048 + (nh + 1) * 512],
                                                 start=(kc == 0), stop=(kc == 7))
                            nc.vector.tensor_copy(vsb[:, nh * 512:(nh + 1) * 512], vq[:])
                        nc.sync.dma_start(vstage[s + mt * 128: s + (mt + 1) * 128, :], vsb[:])

            # ---------------- Phase C: windowed attention ----------------
            with ExitStack() as cd:
                atp = cd.enter_context(tc.tile_pool(name="aT", bufs=1))
                aT = atp.tile([128, 8, HALF], F32R, tag="aT")
                with ExitStack() as cc:
                    qwp = cc.enter_context(tc.tile_pool(name="qw", bufs=2))
                    k2p = cc.enter_context(tc.tile_pool(name="k2w", bufs=2))
                    vwp = cc.enter_context(tc.tile_pool(name="vw", bufs=4))
                    ep = cc.enter_context(tc.tile_pool(name="e", bufs=3))
                    pp = cc.enter_context(tc.tile_pool(name="p", bufs=3))
                    ptp = cc.enter_context(tc.tile_pool(name="pt", bufs=3))
                    sump = cc.enter_context(tc.tile_pool(name="sums", bufs=4))
                    sps = cc.enter_context(tc.tile_pool(name="sps", bufs=3, space="PSUM"))
                    tps2 = cc.enter_context(tc.tile_pool(name="tps2", bufs=2, space="PSUM"))
                    aps = cc.enter_context(tc.tile_pool(name="aps", bufs=3, space="PSUM"))

                    # un-permute staged q/k on load: target row groups
                    # [hA_lo, hA_hi, hB_lo, hB_hi] <- permuted-source offsets
                    UNP = (0, 64, 32, 96)
                    vtiles = {}
                    for w in range(NWIN):
                        qsrc = qrope[:, w * 128:(w + 1) * 128] \
                            .rearrange("(c p) m -> p c m", p=128)
                        qw = qwp.tile([128, 8, 128], F32R, tag="qw")
                        for g, off in enumerate(UNP):
                            nc.sync.dma_start(qw[g * 32:(g + 1) * 32, :, :],
                                              qsrc[off:off + 32])
                        ksrc = k2[:, w, :].rearrange("(c p) j -> p c j", p=128)
                        k2w = k2p.tile([128, 8, 256], F32R, tag="k2w")
                        for g, off in enumerate(UNP):
                            nc.sync.dma_start(k2w[g * 32:(g + 1) * 32, :, :],
                                              ksrc[off:off + 32])
                        for vt in ([w, w + 1] if w == 0 else [w + 1]):
                            v_t = vwp.tile([128, D], BF16, tag="vw")
                            nc.sync.dma_start(v_t[:], vstage[vt * 128:(vt + 1) * 128, :])
                            vtiles[vt] = v_t
                        mslot = 0 if w == 0 else 1
                        for h in range(H):
                            blk, sub = h // 2, h % 2
                            po = sub * 64
                            sp = sps.tile([128, 256], F32, tag="s")
                            nc.tensor.matmul(sp[:], qw[po:po + 64, blk, :],
                                             k2w[po:po + 64, blk, :],
                                             start=True, stop=True)
                            em = ep.tile([128, 256], F32, tag="em")
                            nc.vector.tensor_tensor(em[:], sp[:], mk[:, mslot, :], ADD)
                            ee = ep.tile([128, 256], F32, tag="ee")
                            ssum = sump.tile([128, 1], F32, tag="ss")
                            nc.scalar.activation(ee[:], em[:], EXP, accum_out=ssum[:])
                            rr = sump.tile([128, 1], F32, tag="rr")
                            nc.vector.reciprocal(rr[:], ssum[:])
                            pf = pp.tile([128, 256], BF16, tag="pf")
                            nc.gpsimd.tensor_scalar_mul(pf[:], ee[:], rr[:])
                            ptq = tps2.tile([128, 256], BF16, tag="ptq")
                            nc.tensor.transpose(ptq[:, 0:128], pf[:, 0:128], identb[:])
                            nc.tensor.transpose(ptq[:, 128:256], pf[:, 128:256], identb[:])
                            pt = ptp.tile([128, 256], BF16, tag="pt")
                            nc.scalar.copy(pt[:], ptq[:])
                            ap_ = aps.tile([64, 128], F32, tag="ap")
                            nc.tensor.matmul(ap_[:], vtiles[w][:, h * 64:(h + 1) * 64],
                                             pt[:, 0:128], start=True, stop=False)
                            nc.tensor.matmul(ap_[:], vtiles[w + 1][:, h * 64:(h + 1) * 64],
                                             pt[:, 128:256], start=False, stop=True)
                            nc.scalar.copy(aT[(sub) * 64:(sub) * 64 + 64, blk,
                                              w * 128:(w + 1) * 128], ap_[:])
                        vtiles.pop(w - 1, None)

                # ---------------- Phase D: output projection ----------------
                with ExitStack() as dd:
                    wop = dd.enter_context(tc.tile_pool(name="wo", bufs=1))
                    wo = wop.tile([128, 8, D], F32R, tag="wo")
                    nc.sync.dma_start(wo[:], wout.rearrange("(c p) n -> p c n", p=128))
                    outp = dd.enter_context(tc.tile_pool(name="outsb", bufs=3))
                    ops = dd.enter_context(tc.tile_pool(name="ops", bufs=4, space="PSUM"))
                    for mt in range(16):
                        osb = outp.tile([128, D], F32, tag="o")
                        for nh in range(2):
                            op_ = ops.tile([128, 512], F32, tag="op")
                            for kc in range(8):
                                nc.tensor.matmul(op_[:],
                                                 aT[:, kc, mt * 128:(mt + 1) * 128],
                                                 wo[:, kc, nh * 512:(nh + 1) * 512],
                                                 start=(kc == 0), stop=(kc == 7))
                            nc.vector.tensor_copy(osb[:, nh * 512:(nh + 1) * 512], op_[:])
                        nc.sync.dma_start(out[mt * 128:(mt + 1) * 128, :], osb[:])

    nc.compile()
    return nc


_NC = {}


def _get_nc(reps=1):
    if reps not in _NC:
        _NC[reps] = _build(reps)
    return _NC[reps]


# permutation within each 128-row (2-head) block of head-transposed q/k:
# new row r holds old row ((r//32)%2)*64 + (r%32) + 32*(r//64)
_r = np.arange(128)
_PERM = ((_r // 32) % 2) * 64 + (_r % 32) + 32 * (_r // 64)


def _host_inputs(x, W_qkv, W_out):
    # permute q and k column blocks of W_qkv
    W = np.ascontiguousarray(W_qkv, np.float32).copy()
    for sec in range(2):                     # q section, k section
        for b in range(8):
            base = sec * 1024 + b * 128
            W[:, base:base + 128] = W[:, base + _PERM]

    invf = THETA ** (-(np.arange(0, 64, 2) / 64.0))          # [32]
    rows_f = invf[_r % 32]                                   # [128] freq per row
    # sin tiles are indexed by SOURCE row of the rotate (partner r^64);
    # the destination sign is +1 when the source is a hi-half (r>=64).
    rows_s = np.where(_r < 64, 1.0, -1.0)
    mcol = np.arange(512) % 128
    angC = rows_f[:, None] * (128 + mcol)[None, :]
    angP = rows_f[:, None] * mcol[None, :]
    ropes = np.stack([
        SCALE * np.cos(angC),
        SCALE * (rows_s[:, None] * np.sin(angC)),
        np.cos(angC),
        rows_s[:, None] * np.sin(angC),
        np.cos(angP),
        rows_s[:, None] * np.sin(angP),
    ]).astype(np.float32)

    i = np.arange(128)[:, None]
    jj = np.arange(256)[None, :]
    band = (jj >= i) & (jj <= i + 128)
    maskB = np.where(band, 0.0, NEG).astype(np.float32)
    maskA0 = np.where(band & (jj >= 128), 0.0, NEG).astype(np.float32)

    in_maps = []
    for c in range(NCORES):
        bi, hi = c // 2, c % 2
        xsh = np.empty((NT, D), np.float32)
        if hi == 0:
            xsh[:WS] = 0.0
            xsh[WS:] = x[bi, 0:HALF]
            mA = maskA0
        else:
            xsh[:] = x[bi, HALF - WS: N]
            mA = maskB
        in_maps.append({
            "xs": xsh,
            "wqkv": W,
            "wout": np.ascontiguousarray(W_out, np.float32),
            "ropes": ropes,
            "masks": np.stack([mA, maskB]),
        })
    return in_maps


def kernel(x, W_qkv, W_out):
    x = np.asarray(x, np.float32)
    nc = _get_nc()
    in_maps = _host_inputs(x, W_qkv, W_out)
    res = run_bass_kernel_spmd(nc, in_maps, list(range(NCORES)))
    outf = np.empty((B, N, D), np.float32)
    for c in range(NCORES):
        bi, hi = c // 2, c % 2
        outf[bi, hi * HALF:(hi + 1) * HALF] = res.results[c]["out"]
    return outf


# revision 11
# speedup vs baseline: 1.6267x; 1.6267x over previous
"""LocalMHA (windowed attention, window=128, look_backward=1, RoPE) on 8 TRN2 cores.

Sharding: sequence-parallel, no collectives. Core c handles batch c//2,
sequence half c%2 (2048 query tokens + a 128-token look-backward halo whose
x rows ride along in the core's input shard; zeros at a true sequence start,
where the mask kills the backward keys anyway).

Layout trick: within each 128-row (2-head) block of the head-transposed q/k,
rows are permuted to [hA_d0-31 | hB_d0-31 | hA_d32-63 | hB_d32-63] (host-side
column permutation of W_qkv). The rotate_half partner is then r^64, so RoPE
needs only full-width partition-shifted multiplies (no 32-row fragments), with
the sin sign folded host-side. Scores contract each head's d over two 32-row
chunks (PSUM accumulation) — the dot product is invariant to the d-permutation.

Dtypes: projections and scores in fp32r (full PE rate at free>=256, ~1.6e-4);
attention probabilities and v in bf16 (free-dim-128 matmuls at full rate).

Engine split: PE matmuls/transposes; DVE psum-side elementwise (RoPE muls,
mask-add, evictions); GPSIMD sbuf-side elementwise (RoPE add, normalize);
ACT exp(+row-sum accumulate) and psum evictions that DVE can't absorb.
"""

import numpy as np
from contextlib import ExitStack

import concourse.bacc as bacc
import concourse.tile as tile
import concourse.mybir as mybir
from concourse.bass_utils import run_bass_kernel_spmd
from concourse.masks import make_identity

# Problem shape (hardcoded per contract)
B, N, D = 4, 4096, 1024
H, DH, WS = 16, 64, 128
THETA = 10000.0
N3 = 3 * H * DH            # 3072
NCORES = 8
HALF = N // 2              # 2048 query tokens per core
NT = HALF + WS             # 2176 tokens incl halo window
NWIN = HALF // WS          # 16 query windows
SCALE = DH ** -0.5
NEG = -1.0e9

F32 = mybir.dt.float32
F32R = mybir.dt.float32r
BF16 = mybir.dt.bfloat16
ADD = mybir.AluOpType.add
MUL = mybir.AluOpType.mult
EXP = mybir.ActivationFunctionType.Exp

# token chunks for phases A/B (start, len); 128-aligned, len<=512
CHUNKS = [(0, 512), (512, 512), (1024, 512), (1536, 512), (2048, 128)]


def _rope(nc, tmpp, src_psum, dst, L, rp, ci, si):
    """dst[:, :L] = src*cos + rot64(src)*sin_signed, straight from PSUM.

    Permuted layout: rotate partner of row r is r^64. The sin tile is indexed
    by SOURCE row with the destination's sign folded in host-side, so each
    multiply's two inputs share a base partition (only outputs are shifted).
    The final add runs on GPSIMD (SBUF-only engine), via a plain-f32 view of
    the f32r tile (raw bits through DMA are accepted by the fp32r matmul).
    """
    t1 = tmpp.tile([128, 512], F32, tag="t1")
    nc.vector.tensor_tensor(t1[:, :L], src_psum[:, :L], rp[:, ci, :L], MUL)
    t2 = tmpp.tile([128, 512], F32, tag="t2")
    nc.vector.tensor_tensor(t2[0:64, :L], src_psum[64:128, :L],
                            rp[64:128, si, :L], MUL)
    nc.vector.tensor_tensor(t2[64:128, :L], src_psum[0:64, :L],
                            rp[0:64, si, :L], MUL)
    nc.vector.tensor_tensor(dst[:, :L], t1[:, :L], t2[:, :L], ADD)


def _build(reps=1):
    nc = bacc.Bacc("TRN2", target_bir_lowering=False, debug=False,
                   enable_asserts=False, num_devices=NCORES)

    xs = nc.dram_tensor("xs", [NT, D], F32R, kind="ExternalInput").ap()
    wqkv = nc.dram_tensor("wqkv", [D, N3], F32R, kind="ExternalInput").ap()
    wout = nc.dram_tensor("wout", [D, D], F32R, kind="ExternalInput").ap()
    # 0:qcos 1:qsin 2:kcos_cur 3:ksin_cur 4:kcos_prev 5:ksin_prev
    ropes = nc.dram_tensor("ropes", [6, 128, 512], F32, kind="ExternalInput").ap()
    masks = nc.dram_tensor("masks", [2, 128, 256], F32, kind="ExternalInput").ap()
    out = nc.dram_tensor("out", [HALF, D], F32, kind="ExternalOutput").ap()

    # internal DRAM staging
    qrope = nc.dram_tensor("qrope", [D, HALF], F32R).ap()
    k2 = nc.dram_tensor("k2", [D, NWIN, 2 * WS], F32R).ap()
    vstage = nc.dram_tensor("vstage", [NT, D], BF16).ap()

    with tile.TileContext(nc) as tc:
        with ExitStack() as top:
            constp = top.enter_context(tc.tile_pool(name="const", bufs=1))
            identf = constp.tile([128, 128], F32, tag="idf")
            make_identity(nc, identf[:])
            identb = constp.tile([128, 128], BF16, tag="idb")
            nc.vector.tensor_copy(identb[:], identf[:])
            identr = constp.tile([128, 128], F32R, tag="idr")
            nc.vector.tensor_copy(identr[:], identf[:])
            rp = constp.tile([128, 6, 512], F32, tag="ropes")
            nc.sync.dma_start(rp[:], ropes.rearrange("r p m -> p r m"))
            mk = constp.tile([128, 2, 256], F32, tag="masks")
            nc.sync.dma_start(mk[:], masks.rearrange("r p m -> p r m"))

            rep_ctx = tc.For_i(0, reps, 1) if reps > 1 else ExitStack()
            top.enter_context(rep_ctx)

            # ---------------- Phase A+B: transpose + QKV + RoPE ----------------
            with ExitStack() as ab:
                wp = ab.enter_context(tc.tile_pool(name="wq", bufs=1))
                w_sb = wp.tile([128, 8, N3], F32R, tag="w")
                nc.sync.dma_start(w_sb[:],
                                  wqkv.rearrange("(c p) n -> p c n", p=128))
                xp = ab.enter_context(tc.tile_pool(name="xst", bufs=2))
                xtp = ab.enter_context(tc.tile_pool(name="xT", bufs=2))
                tmpp = ab.enter_context(tc.tile_pool(name="tmp", bufs=3))
                rop = ab.enter_context(tc.tile_pool(name="ro", bufs=4))
                vp = ab.enter_context(tc.tile_pool(name="vsb", bufs=3))
                tps = ab.enter_context(tc.tile_pool(name="tps", bufs=3, space="PSUM"))
                mps = ab.enter_context(tc.tile_pool(name="mps", bufs=4, space="PSUM"))

                for (s, L) in CHUNKS:
                    nmt = L // 128
                    xT = xtp.tile([128, 8, 512], F32R, tag="xT")
                    for mt in range(nmt):
                        xst = xp.tile([128, D], F32R, tag="x")
                        nc.sync.dma_start(xst[:], xs[s + mt * 128: s + (mt + 1) * 128, :])
                        for kc in range(8):
                            tp = tps.tile([128, 128], F32R, tag="tp")
                            nc.tensor.transpose(tp[:], xst[:, kc * 128:(kc + 1) * 128],
                                                identr[:])
                            nc.scalar.copy(xT[:, kc, mt * 128:(mt + 1) * 128], tp[:])

                    # q^T (+rope) for query tokens of this chunk
                    qs = 128 if s == 0 else 0
                    qL = L - qs
                    if qL > 0:
                        for nch in range(8):
                            qp = mps.tile([128, 512], F32, tag="mm")
                            for kc in range(8):
                                nc.tensor.matmul(qp[:, :qL],
                                                 w_sb[:, kc, nch * 128:(nch + 1) * 128],
                                                 xT[:, kc, qs:qs + qL],
                                                 start=(kc == 0), stop=(kc == 7))
                            qf = rop.tile([128, 512], F32R, tag="ro")
                            _rope(nc, tmpp, qp, qf, qL, rp, 0, 1)
                            q0 = s + qs - 128
                            nc.sync.dma_start(
                                qrope[nch * 128:(nch + 1) * 128, q0:q0 + qL],
                                qf[:, :qL])

                    # k^T with both rope phases
                    for nch in range(8):
                        kp = mps.tile([128, 512], F32, tag="mm")
                        for kc in range(8):
                            nc.tensor.matmul(kp[:, :L],
                                             w_sb[:, kc, 1024 + nch * 128: 1024 + (nch + 1) * 128],
                                             xT[:, kc, 0:L],
                                             start=(kc == 0), stop=(kc == 7))
                        cs = 128 if s == 0 else 0     # halo window has no cur slot
                        if L - cs > 0:
                            kcur = rop.tile([128, 512], F32R, tag="ro")
                            _rope(nc, tmpp, kp, kcur, L, rp, 2, 3)
                            w0 = (s + cs) // 128 - 1
                            nw = (L - cs) // 128
                            nc.sync.dma_start(
                                k2[nch * 128:(nch + 1) * 128, w0:w0 + nw, 128:256],
                                kcur[:, cs:L].rearrange("p (w i) -> p w i", i=128))
                        if s + L <= HALF:             # last window has no next
                            kprv = rop.tile([128, 512], F32R, tag="ro")
                            _rope(nc, tmpp, kp, kprv, L, rp, 4, 5)
                            w0 = s // 128
                            nw = L // 128
                            nc.sync.dma_start(
                                k2[nch * 128:(nch + 1) * 128, w0:w0 + nw, 0:128],
                                kprv[:, 0:L].rearrange("p (w i) -> p w i", i=128))

                    # v in natural layout, bf16
                    for mt in range(nmt):
                        vsb = vp.tile([128, D], BF16, tag="v")
                        for nh in range(2):
                            vq = mps.tile([128, 512], F32, tag="mm")
                            for kc in range(8):
                                nc.tensor.matmul(vq[:],
                                                 xT[:, kc, mt * 128:(mt + 1) * 128],
                                                 w_sb[:, kc, 2048 + nh * 512: 2048 + (nh + 1) * 512],
                                                 start=(kc == 0), stop=(kc == 7))
                            nc.vector.tensor_copy(vsb[:, nh * 512:(nh + 1) * 512], vq[:])
                        nc.sync.dma_start(vstage[s + mt * 128: s + (mt + 1) * 128, :], vsb[:])

            # ---------------- Phase C: windowed attention ----------------
            with ExitStack() as cd:
                atp = cd.enter_context(tc.tile_pool(name="aT", bufs=1))
                aT = atp.tile([128, 8, HALF], F32R, tag="aT")
                with ExitStack() as cc:
                    qwp = cc.enter_context(tc.tile_pool(name="qw", bufs=2))
                    k2p = cc.enter_context(tc.tile_pool(name="k2w", bufs=2))
                    vwp = cc.enter_context(tc.tile_pool(name="vw", bufs=4))
                    ep = cc.enter_context(tc.tile_pool(name="e", bufs=3))
                    pp = cc.enter_context(tc.tile_pool(name="p", bufs=3))
                    ptp = cc.enter_context(tc.tile_pool(name="pt", bufs=3))
                    sump = cc.enter_context(tc.tile_pool(name="sums", bufs=4))
                    sps = cc.enter_context(tc.tile_pool(name="sps", bufs=3, space="PSUM"))
                    tps2 = cc.enter_context(tc.tile_pool(name="tps2", bufs=2, space="PSUM"))
                    aps = cc.enter_context(tc.tile_pool(name="aps", bufs=3, space="PSUM"))

                    # un-permute staged q/k on load: target row groups
                    # [hA_lo, hA_hi, hB_lo, hB_hi] <- permuted-source offsets
                    UNP = (0, 64, 32, 96)
                    vtiles = {}
                    for w in range(NWIN):
                        qsrc = qrope[:, w * 128:(w + 1) * 128] \
                            .rearrange("(c p) m -> p c m", p=128)
                        qw = qwp.tile([128, 8, 128], F32R, tag="qw")
                        for g, off in enumerate(UNP):
                            nc.sync.dma_start(qw[g * 32:(g + 1) * 32, :, :],
                                              qsrc[off:off + 32])
                        ksrc = k2[:, w, :].rearrange("(c p) j -> p c j", p=128)
                        k2w = k2p.tile([128, 8, 256], F32R, tag="k2w")
                        for g, off in enumerate(UNP):
                            nc.sync.dma_start(k2w[g * 32:(g + 1) * 32, :, :],
                                              ksrc[off:off + 32])
                        for vt in ([w, w + 1] if w == 0 else [w + 1]):
                            v_t = vwp.tile([128, D], BF16, tag="vw")
                            nc.sync.dma_start(v_t[:], vstage[vt * 128:(vt + 1) * 128, :])
                            vtiles[vt] = v_t
                        mslot = 0 if w == 0 else 1
                        for h in range(H):
                            blk, sub = h // 2, h % 2
                            po = sub * 64
                            sp = sps.tile([128, 256], F32, tag="s")
                            nc.tensor.matmul(sp[:], qw[po:po + 64, blk, :],
                                             k2w[po:po + 64, blk, :],
                                             start=True, stop=True)
                            em = ep.tile([128, 256], F32, tag="em")
                            nc.vector.tensor_tensor(em[:], sp[:], mk[:, mslot, :], ADD)
                            ee = ep.tile([128, 256], F32, tag="ee")
                            ssum = sump.tile([128, 1], F32, tag="ss")
                            nc.scalar.activation(ee[:], em[:], EXP, accum_out=ssum[:])
                            rr = sump.tile([128, 1], F32, tag="rr")
                            nc.vector.reciprocal(rr[:], ssum[:])
                            pf = pp.tile([128, 256], BF16, tag="pf")
                            nc.vector.tensor_scalar_mul(pf[:], ee[:], rr[:])
                            ptq = tps2.tile([128, 256], BF16, tag="ptq")
                            nc.tensor.transpose(ptq[:, 0:128], pf[:, 0:128], identb[:])
                            nc.tensor.transpose(ptq[:, 128:256], pf[:, 128:256], identb[:])
                            pt = ptp.tile([128, 256], BF16, tag="pt")
                            nc.scalar.copy(pt[:], ptq[:])
                            ap_ = aps.tile([64, 128], F32, tag="ap")
                            nc.tensor.matmul(ap_[:], vtiles[w][:, h * 64:(h + 1) * 64],
                                             pt[:, 0:128], start=True, stop=False)
                            nc.tensor.matmul(ap_[:], vtiles[w + 1][:, h * 64:(h + 1) * 64],
                                             pt[:, 128:256], start=False, stop=True)
                            nc.scalar.copy(aT[(sub) * 64:(sub) * 64 + 64, blk,
                                              w * 128:(w + 1) * 128], ap_[:])
                        vtiles.pop(w - 1, None)

                # ---------------- Phase D: output projection ----------------
                with ExitStack() as dd:
                    wop = dd.enter_context(tc.tile_pool(name="wo", bufs=1))
                    wo = wop.tile([128, 8, D], F32R, tag="wo")
                    nc.sync.dma_start(wo[:], wout.rearrange("(c p) n -> p c n", p=128))
                    outp = dd.enter_context(tc.tile_pool(name="outsb", bufs=3))
                    ops = dd.enter_context(tc.tile_pool(name="ops", bufs=4, space="PSUM"))
                    for mt in range(16):
                        osb = outp.tile([128, D], F32, tag="o")
                        for nh in range(2):
                            op_ = ops.tile([128, 512], F32, tag="op")
                            for kc in range(8):
                                nc.tensor.matmul(op_[:],
                                                 aT[:, kc, mt * 128:(mt + 1) * 128],
                                                 wo[:, kc, nh * 512:(nh + 1) * 512],
                                                 start=(kc == 0), stop=(kc == 7))
                            nc.vector.tensor_copy(osb[:, nh * 512:(nh + 1) * 512], op_[:])
                        nc.sync.dma_start(out[mt * 128:(mt + 1) * 128, :], osb[:])

    nc.compile()
    return nc


_NC = {}


def _get_nc(reps=1):
    if reps not in _NC:
        _NC[reps] = _build(reps)
    return _NC[reps]


# permutation within each 128-row (2-head) block of head-transposed q/k:
# new row r holds old row ((r//32)%2)*64 + (r%32) + 32*(r//64)
_r = np.arange(128)
_PERM = ((_r // 32) % 2) * 64 + (_r % 32) + 32 * (_r // 64)


def _host_inputs(x, W_qkv, W_out):
    # permute q and k column blocks of W_qkv
    W = np.ascontiguousarray(W_qkv, np.float32).copy()
    for sec in range(2):                     # q section, k section
        for b in range(8):
            base = sec * 1024 + b * 128
            W[:, base:base + 128] = W[:, base + _PERM]

    invf = THETA ** (-(np.arange(0, 64, 2) / 64.0))          # [32]
    rows_f = invf[_r % 32]                                   # [128] freq per row
    # sin tiles are indexed by SOURCE row of the rotate (partner r^64);
    # the destination sign is +1 when the source is a hi-half (r>=64).
    rows_s = np.where(_r < 64, 1.0, -1.0)
    mcol = np.arange(512) % 128
    angC = rows_f[:, None] * (128 + mcol)[None, :]
    angP = rows_f[:, None] * mcol[None, :]
    ropes = np.stack([
        SCALE * np.cos(angC),
        SCALE * (rows_s[:, None] * np.sin(angC)),
        np.cos(angC),
        rows_s[:, None] * np.sin(angC),
        np.cos(angP),
        rows_s[:, None] * np.sin(angP),
    ]).astype(np.float32)

    i = np.arange(128)[:, None]
    jj = np.arange(256)[None, :]
    band = (jj >= i) & (jj <= i + 128)
    maskB = np.where(band, 0.0, NEG).astype(np.float32)
    maskA0 = np.where(band & (jj >= 128), 0.0, NEG).astype(np.float32)

    in_maps = []
    for c in range(NCORES):
        bi, hi = c // 2, c % 2
        xsh = np.empty((NT, D), np.float32)
        if hi == 0:
            xsh[:WS] = 0.0
            xsh[WS:] = x[bi, 0:HALF]
            mA = maskA0
        else:
            xsh[:] = x[bi, HALF - WS: N]
            mA = maskB
        in_maps.append({
            "xs": xsh,
            "wqkv": W,
            "wout": np.ascontiguousarray(W_out, np.float32),
            "ropes": ropes,
            "masks": np.stack([mA, maskB]),
        })
    return in_maps


def kernel(x, W_qkv, W_out):
    x = np.asarray(x, np.float32)
    nc = _get_nc()
    in_maps = _host_inputs(x, W_qkv, W_out)
    res = run_bass_kernel_spmd(nc, in_maps, list(range(NCORES)))
    outf = np.empty((B, N, D), np.float32)
    for c in range(NCORES):
        bi, hi = c // 2, c % 2
        outf[bi, hi * HALF:(hi + 1) * HALF] = res.results[c]["out"]
    return outf
